# revision 42
# baseline (speedup 1.0000x reference)
"""Dilated (dil=2) 7x7 window self-attention, 4 heads x 32 dim, on 8 trn2 cores.

Strategy: spatial sharding over image rows (12 rows/core, 6-row halo).
Inside each core, the dilation-2 window decomposes the image into 4
cosets (row/col parity); within a coset the attention is a dense 7x7
window on a 48x48 grid (12 key rows x 48 cols vs 6 query rows x 48
cols per (batch, coset) block).  All tensors are channel-major
[128, pix]; logits are computed transposed [nk, nq] per block so both
attention einsums are matmuls without transposes.  All matmuls run in
bf16 (1 PE cycle/row), accumulation in fp32 PSUM.

Key chunking is by full-height COLUMN slabs: a chunk is all 12 key
rows x ~10 key cols (<=120 partitions).  A chunk's in-window queries
are all 6 query rows x (cols +-3) — so each query is touched by only
~1.5 chunks, vs 4 with row-pair chunking.  That shrinks the streamed
free size of the logits / attn@V / denominator matmuls and the exp /
mask element count by ~2.6x.

  K^T Q  : per (chunk, head) one [32,120]-lhsT matmul (keys via a 2D
           strided AP over the row-major key layout).
  softmax: unnormalized exp (no max-subtraction; logits are tiny) with
           the key-pixel mask bias (-60 per masked key) folded into the
           ACT exp bias, written directly in bf16; out-of-window pairs
           zeroed by one bf16 multiply per (chunk, head-pair) with a
           precomputed 0/1 window tensor; softmax denominators come
           from a ones-weight matmul and are divided out after attn@V.
  attn@V : col-tiled (4 heads) matmuls accumulating into a 2D-strided
           PSUM region (query rows x cols), PSUM zeroed per block by a
           rank-1 zero matmul with start=True.
"""

import numpy as np

HEADS, D, WIN, DIL = 4, 32, 7, 2
B, C, H, W = 2, 128, 96, 96
CORES, RPC = 8, 12
CR, KR, W2 = 6, 12, 48            # coset query rows / key rows (halo) / cols
NQ, NK = CR * W2, KR * W2         # 288, 576
NBLK = B * 4                      # (batch, coset) blocks per core
SCALE = float(1.0 / np.sqrt(D))
MBIAS = -60.0

# full-height column chunks: (key col0, ncols, query col lo, query width)
CHUNKS = [(0, 10, 0, 13), (10, 10, 7, 16), (20, 10, 17, 16),
          (30, 10, 27, 16), (40, 8, 37, 11)]
NC = len(CHUNKS)
SLOT = 96                         # attnT slot per chunk (max 6*16)
CHUNK_PC = [12 * c[1] for c in CHUNKS]            # keys per chunk
CHUNK_OFF = [sum(CHUNK_PC[:i]) for i in range(NC)]  # chunk-major key offset

_prog = None


def _build_program():
    import concourse.bass as bass
    import concourse.tile as tile
    from concourse import mybir

    nc = bass.Bass("TRN2", target_bir_lowering=False, debug=False,
                   num_devices=CORES)
    f32 = mybir.dt.float32
    bf = mybir.dt.bfloat16
    xc = nc.dram_tensor("xc", [128, NBLK * NK], bf, kind="ExternalInput").ap()
    xqi = nc.dram_tensor("xq", [128, NBLK * NQ], bf,
                         kind="ExternalInput").ap()
    mb_i = nc.dram_tensor("mb", [128, NBLK * NC], f32,
                          kind="ExternalInput").ap()
    winm = nc.dram_tensor("winm", [128, 4 * NC * SLOT], bf,
                          kind="ExternalInput").ap()
    wq = nc.dram_tensor("wq", [128, 128], bf, kind="ExternalInput").ap()
    wk = nc.dram_tensor("wk", [128, 128], bf, kind="ExternalInput").ap()
    wv = nc.dram_tensor("wv", [128, 128], bf, kind="ExternalInput").ap()
    wp = nc.dram_tensor("wp", [128, 128], bf, kind="ExternalInput").ap()
    out = nc.dram_tensor("out", [128, NBLK * NQ], f32,
                         kind="ExternalOutput").ap()

    def keys_ap(t, base, c):
        """contiguous chunk-major key slice (host pre-permutes keys)."""
        return t[:, base + CHUNK_OFF[c]: base + CHUNK_OFF[c] + CHUNK_PC[c]]

    with tile.TileContext(nc) as tc:
        with tc.tile_pool(name="cst", bufs=1) as cst, \
             tc.tile_pool(name="big", bufs=1) as big, \
             tc.tile_pool(name="qk", bufs=1) as qkp, \
             tc.tile_pool(name="vt", bufs=2) as vtp, \
             tc.tile_pool(name="att", bufs=2) as attp, \
             tc.tile_pool(name="oev", bufs=3) as oev, \
             tc.tile_pool(name="psL", bufs=2, space="PSUM") as psL, \
             tc.tile_pool(name="psO", bufs=1, space="PSUM") as psO, \
             tc.tile_pool(name="psP", bufs=2, space="PSUM") as psP:

            w_q = cst.tile([128, 128], bf)
            nc.gpsimd.dma_start(out=w_q[:], in_=wq[:])
            w_k = cst.tile([128, 128], bf)
            nc.gpsimd.dma_start(out=w_k[:], in_=wk[:])
            w_v = cst.tile([128, 128], bf)
            nc.gpsimd.dma_start(out=w_v[:], in_=wv[:])
            w_p = cst.tile([128, 128], bf)
            nc.gpsimd.dma_start(out=w_p[:], in_=wp[:])

            # block-0 inputs + masks first so compute starts early, then
            # the rest of X/Xq as bulk transfers.
            X = big.tile([128, NBLK * NK], bf)
            Xq = big.tile([128, NBLK * NQ], bf)
            WM = big.tile([128, 4 * NC * SLOT], bf)   # win mask, (h,c)-major
            mbias = cst.tile([128, NBLK * NC], f32)
            nc.gpsimd.dma_start(out=Xq[:, :NQ], in_=xqi[:, :NQ])
            nc.gpsimd.dma_start(out=X[:, :NK], in_=xc[:, :NK])
            nc.scalar.dma_start(out=mbias[:], in_=mb_i[:])
            nc.sync.dma_start(out=WM[:], in_=winm[:])
            nc.gpsimd.dma_start(out=Xq[:, NQ:2 * NQ], in_=xqi[:, NQ:2 * NQ])
            nc.gpsimd.dma_start(out=X[:, NK:2 * NK], in_=xc[:, NK:2 * NK])
            nc.gpsimd.dma_start(out=Xq[:, 2 * NQ:], in_=xqi[:, 2 * NQ:])
            nc.gpsimd.dma_start(out=X[:, 2 * NK:], in_=xc[:, 2 * NK:])

            for _ in range(2):
                pL0 = psL.tile([128, 1024], f32, tag="psL")
                nc.vector.memset(pL0[:], 0.0)

            ones = cst.tile([128, 32], bf)
            nc.vector.memset(ones[:], 1.0)
            zrow = cst.tile([128, 128], bf)
            nc.vector.memset(zrow[:], 0.0)

            # Q and K channel-major projections, pipelined per block.
            Q = qkp.tile([128, NBLK * NQ], bf)
            K = qkp.tile([128, NBLK * NK], bf)

            def proj(blk):
                pq = psP.tile([128, 512], f32, tag="psP")
                nc.tensor.matmul(out=pq[:, :NQ], lhsT=w_q[:],
                                 rhs=Xq[:, blk * NQ:(blk + 1) * NQ],
                                 start=True, stop=True)
                if blk % 2:
                    nc.scalar.copy(out=Q[:, blk * NQ:(blk + 1) * NQ], in_=pq[:, :NQ])
                else:
                    nc.vector.tensor_copy(Q[:, blk * NQ:(blk + 1) * NQ], pq[:, :NQ])
                for half in range(2):
                    pk = psP.tile([128, 512], f32, tag="psP")
                    sl = slice(blk * NK + half * NQ, blk * NK + (half + 1) * NQ)
                    nc.tensor.matmul(out=pk[:, :NQ], lhsT=w_k[:], rhs=X[:, sl],
                                     start=True, stop=True)
                    if half:
                        nc.scalar.copy(out=K[:, sl], in_=pk[:, :NQ])
                    else:
                        nc.vector.tensor_copy(K[:, sl], pk[:, :NQ])

            proj(0)

            pending_tail = [None, None]

            def flush_tail(idx):
                if pending_tail[idx] is not None:
                    pending_tail[idx]()
                    pending_tail[idx] = None

            for blk in range(NBLK):
                # --- V^T production: one matmul per column chunk ---
                VT = vtp.tile([128, NC * 128], bf, tag="vt")
                pv = psP.tile([128, 512], f32, tag="psP")
                for c in range(4):
                    pc = 12 * CHUNKS[c][1]
                    nc.tensor.matmul(
                        out=pv[:pc, c * 128:(c + 1) * 128],
                        lhsT=keys_ap(X, blk * NK, c),
                        rhs=w_v[:], start=True, stop=True)
                nc.vector.tensor_copy(VT[:, :512], pv[:, :512])
                pv2 = psP.tile([128, 512], f32, tag="psP")
                nc.tensor.matmul(
                    out=pv2[:96, :128],
                    lhsT=keys_ap(X, blk * NK, 4),
                    rhs=w_v[:], start=True, stop=True)
                nc.scalar.copy(out=VT[:96, 512:640], in_=pv2[:96, :128])

                attnT = attp.tile([128, 4 * NC * SLOT], bf, tag="att")
                pO = psO.tile([128, 512], f32, tag="psO")
                pS = psO.tile([128, 512], f32, tag="psS")
                if blk == 0:
                    # later blocks are zeroed inside the previous tail
                    nc.vector.memset(pO[:, :NQ], 0.0)
                    nc.vector.memset(pS[:, :NQ], 0.0)

                def unit(c, hp, blk=blk, attnT=attnT):
                    """logits + exp + mask for heads {2hp,2hp+1}, chunk c."""
                    kc0, ncols, qlo, qw = CHUNKS[c]
                    pc, nf = 12 * ncols, CR * qw
                    pL = psL.tile([128, 1024], f32, tag="psL")
                    qv = Q[:, blk * NQ:(blk + 1) * NQ].rearrange(
                        "p (r w) -> p r w", r=CR)
                    for hh in range(2):
                        h = 2 * hp + hh
                        nc.tensor.matmul(
                            out=pL[0:pc, 512 * hh: 512 * hh + nf],
                            lhsT=keys_ap(K[32 * h:32 * h + 32], blk * NK, c),
                            rhs=qv[32 * h:32 * h + 32, :, qlo:qlo + qw],
                            start=True, stop=True,
                            tile_position=(32 * h, 0),
                        )
                    src = pL[:pc].rearrange("p (h n) -> p h n", h=2)[:, :, :nf]
                    att2 = attnT[:pc].rearrange("p (h c n) -> p h c n",
                                                h=4, c=NC)[:, 2 * hp:2 * hp + 2,
                                                           c, :nf]
                    nc.scalar.activation(
                        out=att2, in_=src,
                        func=mybir.ActivationFunctionType.Exp,
                        bias=mbias[0:pc, blk * NC + c: blk * NC + c + 1],
                        scale=SCALE,
                    )
                    wm2 = WM[:pc].rearrange("p (h c n) -> p h c n",
                                            h=4, c=NC)[:, 2 * hp:2 * hp + 2,
                                                       c, :nf]
                    nc.vector.tensor_mul(out=att2, in0=att2, in1=wm2)

                def phase2(c, hp, blk=blk, attnT=attnT, pO=pO, pS=pS, VT=VT):
                    kc0, ncols, qlo, qw = CHUNKS[c]
                    pc, nf = 12 * ncols, CR * qw
                    po_v = pO[:, :NQ].rearrange("p (r w) -> p r w", r=CR)
                    ps_v = pS[:, :NQ].rearrange("p (r w) -> p r w", r=CR)
                    for hh in range(2):
                        h = 2 * hp + hh
                        rhs = attnT[0:pc, (h * NC + c) * SLOT:
                                    (h * NC + c) * SLOT + nf]
                        nc.tensor.matmul(
                            out=po_v[32 * h:32 * h + 32, :, qlo:qlo + qw],
                            lhsT=VT[0:pc, c * 128 + 32 * h:
                                    c * 128 + 32 * h + 32],
                            rhs=rhs, start=False,
                            stop=(c == NC - 1 and hp == 1),
                            tile_position=(0, 32 * h),
                        )
                        nc.tensor.matmul(
                            out=ps_v[32 * h:32 * h + 32, :, qlo:qlo + qw],
                            lhsT=ones[0:pc, :],
                            rhs=rhs, start=False,
                            stop=(c == NC - 1 and hp == 1),
                            tile_position=(0, 32 * h),
                        )

                units = [(c, hp) for c in range(NC) for hp in range(2)]
                for i, (c, hp) in enumerate(units):
                    unit(c, hp)
                    if i == 1:
                        flush_tail(0)     # normalize chain of prev block
                    if i == 5:
                        flush_tail(1)     # projection + store of prev block
                    if i == 6 and blk + 1 < NBLK:
                        proj(blk + 1)
                    if i >= 3:
                        phase2(*units[i - 3])
                for j in (7, 8, 9):
                    phase2(*units[j])

                onrm = oev.tile([128, NQ], bf, tag="onrm")

                def tail_a(blk=blk, pO=pO, pS=pS, onrm=onrm):
                    # 1/S = exp(-ln S) on the scalar engine: ~2.5x cheaper
                    # than DVE reciprocal and off the loaded vector engine.
                    lns = oev.tile([128, NQ], f32, tag="lns")
                    nc.scalar.activation(
                        out=lns[:], in_=pS[:, :NQ],
                        func=mybir.ActivationFunctionType.Ln)
                    nc.vector.memset(pS[:, :NQ], 0.0)   # ready for next blk
                    rcp = oev.tile([128, NQ], f32, tag="rcp")
                    nc.scalar.activation(
                        out=rcp[:], in_=lns[:],
                        func=mybir.ActivationFunctionType.Exp, scale=-1.0)
                    nc.vector.tensor_mul(out=onrm[:], in0=pO[:, :NQ],
                                         in1=rcp[:])
                    nc.vector.memset(pO[:, :NQ], 0.0)   # ready for next blk

                def tail_b(blk=blk, onrm=onrm):
                    pF = psP.tile([128, 512], f32, tag="psP")
                    nc.tensor.matmul(out=pF[:, :NQ], lhsT=w_p[:], rhs=onrm[:],
                                     start=True, stop=True)
                    osb = oev.tile([128, NQ], f32, tag="osb")
                    nc.vector.tensor_copy(osb[:], pF[:, :NQ])
                    nc.gpsimd.dma_start(out=out[:, blk * NQ:(blk + 1) * NQ],
                                        in_=osb[:])

                pending_tail[0] = tail_a
                pending_tail[1] = tail_b
            flush_tail(0)
            flush_tail(1)

    _split_multi_waits(nc)
    return nc


def _split_multi_waits(nc):
    """This walrus build rejects >1 sem wait per instruction: move extra
    waits onto dedicated single-wait NoOps inserted just before."""
    import copy
    from concourse import mybir

    tmpl = nc.sync.nop(nofuse=True, hint="wsplit_template").ins
    bb0 = nc.cur_bb.bb
    bb0.instructions = [i for i in bb0.instructions if i.name != tmpl.name]
    tmpl = copy.deepcopy(tmpl)

    ctr = 0
    for f in nc.m.functions:
        for bb in f.blocks:
            insts = list(bb.instructions)
            new, changed = [], False
            for inst in insts:
                si = getattr(inst, "sync_info", None)
                waits = list(si.on_wait) if si is not None and si.on_wait else []
                if len(waits) > 1:
                    for w in waits[:-1]:
                        ctr += 1
                        nop = copy.deepcopy(tmpl)
                        nop.name = f"I-wsplit{ctr}"
                        nop.engine = inst.engine
                        nop.sync_info = mybir.SyncInfo(on_wait=[w], on_update=[])
                        new.append(nop)
                    si.on_wait = [waits[-1]]
                    changed = True
                new.append(inst)
            if changed:
                bb.instructions = new


def _chunk_key_index(c):
    """key indices (r*48+kc) of chunk c, row-major, as used on-device."""
    kc0, ncols, _, _ = CHUNKS[c]
    rr = np.arange(KR)[:, None]
    cc = np.arange(kc0, kc0 + ncols)[None, :]
    return (rr * W2 + cc).reshape(-1)


def _host_prep(x, m):
    import ml_dtypes
    bfd = ml_dtypes.bfloat16
    key_perm = np.concatenate([_chunk_key_index(c) for c in range(NC)])
    xs, xqs, ms = [], [], []
    for k in range(CORES):
        r0 = 12 * k - 6
        xpad = np.zeros((B, C, 24, W), np.float32)
        mpad = np.zeros((B, 1, 24, W), np.int32)
        lo, hi = max(0, r0), min(H, r0 + 24)
        xpad[:, :, lo - r0:hi - r0] = x[:, :, lo:hi]
        mpad[:, :, lo - r0:hi - r0] = m[:, :, lo:hi]
        xcs = xpad.reshape(B, C, KR, 2, W2, 2).transpose(1, 0, 3, 5, 2, 4)
        xcs = xcs.reshape(C, NBLK, NK)
        xq = np.ascontiguousarray(
            xcs[:, :, 144:144 + NQ].reshape(C, NBLK * NQ).astype(bfd))
        xck = np.ascontiguousarray(
            xcs[:, :, key_perm].reshape(C, NBLK * NK).astype(bfd))
        mc = mpad.reshape(B, 1, KR, 2, W2, 2).transpose(1, 0, 3, 5, 2, 4)
        mc = mc.reshape(B, 4, NK)
        mb = np.zeros((128, NBLK * NC), np.float32)
        for b in range(B):
            for cs in range(4):
                for c in range(NC):
                    idx = _chunk_key_index(c)
                    mb[:len(idx), (b * 4 + cs) * NC + c] = np.where(
                        mc[b, cs, idx] > 0, 0.0, MBIAS)
        xs.append(xck)
        xqs.append(xq)
        ms.append(np.ascontiguousarray(mb))
    return xs, xqs, ms


def _host_win():
    """[128, 4*NC*SLOT] bf16: 0/1 win mask, chunk-key partition order,
    4 identical head copies; slot layout (qr, qc-qlo)."""
    import ml_dtypes
    wm = np.zeros((128, 4, NC, SLOT), np.float32)
    for c, (kc0, ncols, qlo, qw) in enumerate(CHUNKS):
        rr = np.arange(KR)[:, None]          # key rows
        cc = np.arange(kc0, kc0 + ncols)[None, :]
        kr = np.repeat(rr, ncols, 1).reshape(-1)[:, None]   # [pc,1]
        kc = np.repeat(cc, KR, 0).reshape(-1)[:, None]
        qr = np.arange(CR)[None, :, None]
        qc = np.arange(qlo, qlo + qw)[None, None, :]
        win = ((kr[:, :, None] - qr >= 0) & (kr[:, :, None] - qr <= 6)
               & (np.abs(kc[:, :, None] - qc) <= 3))
        pc, nf = KR * ncols, CR * qw
        wm[:pc, :, c, :nf] = win.reshape(pc, nf)[:, None, :]
    return np.ascontiguousarray(
        wm.reshape(128, 4 * NC * SLOT).astype(ml_dtypes.bfloat16))


def kernel(x, m, Wq, Wk, Wv, Wp):
    global _prog
    import ml_dtypes
    from concourse.bass_utils import run_bass_kernel_spmd

    bfd = ml_dtypes.bfloat16
    x = np.asarray(x, dtype=np.float32)
    m = np.asarray(m, dtype=np.int32)
    if _prog is None:
        _prog = _build_program()
    nc = _prog

    xs, xqs, ms = _host_prep(x, m)
    wmask = _host_win()
    base = {
        "winm": wmask,
        "wq": np.ascontiguousarray(np.asarray(Wq, np.float32).T.astype(bfd)),
        "wk": np.ascontiguousarray(np.asarray(Wk, np.float32).T.astype(bfd)),
        "wv": np.ascontiguousarray(np.asarray(Wv, np.float32).T.astype(bfd)),
        "wp": np.ascontiguousarray(np.asarray(Wp, np.float32).T.astype(bfd)),
    }
    in_maps = [{**base, "xc": xs[k], "xq": xqs[k], "mb": ms[k]}
               for k in range(CORES)]
    res = run_bass_kernel_spmd(nc, in_maps, list(range(CORES)))

    full = np.zeros((B, C, H, W), np.float32)
    for k in range(CORES):
        oc = res.results[k]["out"].reshape(C, B, 2, 2, CR, W2)
        o = oc.transpose(1, 0, 4, 2, 5, 3).reshape(B, C, 12, 96)
        full[:, :, 12 * k:12 * k + 12, :] = o
    return full


# revision 43
# speedup vs baseline: 1.0246x; 1.0246x over previous
"""Dilated (dil=2) 7x7 window self-attention, 4 heads x 32 dim, on 8 trn2 cores.

Strategy: spatial sharding over image rows (12 rows/core, 6-row halo).
Inside each core, the dilation-2 window decomposes the image into 4
cosets (row/col parity); within a coset the attention is a dense 7x7
window on a 48x48 grid (12 key rows x 48 cols vs 6 query rows x 48
cols per (batch, coset) block).  All tensors are channel-major
[128, pix]; logits are computed transposed [nk, nq] per block so both
attention einsums are matmuls without transposes.  All matmuls run in
bf16 (1 PE cycle/row), accumulation in fp32 PSUM.

Key chunking is by full-height COLUMN slabs: a chunk is all 12 key
rows x ~10 key cols (<=120 partitions).  A chunk's in-window queries
are all 6 query rows x (cols +-3) — so each query is touched by only
~1.5 chunks, vs 4 with row-pair chunking.  That shrinks the streamed
free size of the logits / attn@V / denominator matmuls and the exp /
mask element count by ~2.6x.

  K^T Q  : per (chunk, head) one [32,120]-lhsT matmul (keys via a 2D
           strided AP over the row-major key layout).
  softmax: unnormalized exp (no max-subtraction; logits are tiny) with
           the key-pixel mask bias (-60 per masked key) folded into the
           ACT exp bias, written directly in bf16; out-of-window pairs
           zeroed by one bf16 multiply per (chunk, head-pair) with a
           precomputed 0/1 window tensor; softmax denominators come
           from a ones-weight matmul and are divided out after attn@V.
  attn@V : col-tiled (4 heads) matmuls accumulating into a 2D-strided
           PSUM region (query rows x cols), PSUM zeroed per block by a
           rank-1 zero matmul with start=True.
"""

import numpy as np

HEADS, D, WIN, DIL = 4, 32, 7, 2
B, C, H, W = 2, 128, 96, 96
CORES, RPC = 8, 12
CR, KR, W2 = 6, 12, 48            # coset query rows / key rows (halo) / cols
NQ, NK = CR * W2, KR * W2         # 288, 576
NBLK = B * 4                      # (batch, coset) blocks per core
SCALE = float(1.0 / np.sqrt(D))
MBIAS = -60.0

# full-height column chunks: (key col0, ncols, query col lo, query width)
CHUNKS = [(0, 10, 0, 13), (10, 10, 7, 16), (20, 10, 17, 16),
          (30, 10, 27, 16), (40, 8, 37, 11)]
NC = len(CHUNKS)
SLOT = 96                         # attnT slot per chunk (max 6*16)
CHUNK_PC = [12 * c[1] for c in CHUNKS]            # keys per chunk
CHUNK_OFF = [sum(CHUNK_PC[:i]) for i in range(NC)]  # chunk-major key offset

_prog = None


def _build_program():
    import concourse.bass as bass
    import concourse.tile as tile
    from concourse import mybir

    nc = bass.Bass("TRN2", target_bir_lowering=False, debug=False,
                   num_devices=CORES)
    f32 = mybir.dt.float32
    bf = mybir.dt.bfloat16
    xc = nc.dram_tensor("xc", [128, NBLK * NK], bf, kind="ExternalInput").ap()
    xqi = nc.dram_tensor("xq", [128, NBLK * NQ], bf,
                         kind="ExternalInput").ap()
    mb_i = nc.dram_tensor("mb", [128, NBLK * NC], f32,
                          kind="ExternalInput").ap()
    winm = nc.dram_tensor("winm", [128, 4 * NC * SLOT], bf,
                          kind="ExternalInput").ap()
    wq = nc.dram_tensor("wq", [128, 128], bf, kind="ExternalInput").ap()
    wk = nc.dram_tensor("wk", [128, 128], bf, kind="ExternalInput").ap()
    wv = nc.dram_tensor("wv", [128, 128], bf, kind="ExternalInput").ap()
    wp = nc.dram_tensor("wp", [128, 128], bf, kind="ExternalInput").ap()
    out = nc.dram_tensor("out", [128, NBLK * NQ], f32,
                         kind="ExternalOutput").ap()

    def keys_ap(t, base, c):
        """contiguous chunk-major key slice (host pre-permutes keys)."""
        return t[:, base + CHUNK_OFF[c]: base + CHUNK_OFF[c] + CHUNK_PC[c]]

    with tile.TileContext(nc) as tc:
        with tc.tile_pool(name="cst", bufs=1) as cst, \
             tc.tile_pool(name="big", bufs=1) as big, \
             tc.tile_pool(name="qk", bufs=1) as qkp, \
             tc.tile_pool(name="vt", bufs=2) as vtp, \
             tc.tile_pool(name="att", bufs=2) as attp, \
             tc.tile_pool(name="oev", bufs=3) as oev, \
             tc.tile_pool(name="psL", bufs=2, space="PSUM") as psL, \
             tc.tile_pool(name="psO", bufs=1, space="PSUM") as psO, \
             tc.tile_pool(name="psP", bufs=2, space="PSUM") as psP:

            w_q = cst.tile([128, 128], bf)
            nc.gpsimd.dma_start(out=w_q[:], in_=wq[:])
            w_k = cst.tile([128, 128], bf)
            nc.gpsimd.dma_start(out=w_k[:], in_=wk[:])
            w_v = cst.tile([128, 128], bf)
            nc.gpsimd.dma_start(out=w_v[:], in_=wv[:])
            w_p = cst.tile([128, 128], bf)
            nc.gpsimd.dma_start(out=w_p[:], in_=wp[:])

            # block-0 inputs + masks first so compute starts early, then
            # the rest of X/Xq as bulk transfers.
            X = big.tile([128, NBLK * NK], bf)
            Xq = big.tile([128, NBLK * NQ], bf)
            WM = big.tile([128, 4 * NC * SLOT], bf)   # win mask, (h,c)-major
            mbias = cst.tile([128, NBLK * NC], f32)
            nc.scalar.dma_start(out=Xq[:, :NQ], in_=xqi[:, :NQ])
            nc.sync.dma_start(out=X[:, :NK], in_=xc[:, :NK])
            nc.scalar.dma_start(out=mbias[:], in_=mb_i[:])
            nc.sync.dma_start(out=WM[:], in_=winm[:])
            nc.gpsimd.dma_start(out=Xq[:, NQ:2 * NQ], in_=xqi[:, NQ:2 * NQ])
            nc.gpsimd.dma_start(out=X[:, NK:2 * NK], in_=xc[:, NK:2 * NK])
            nc.gpsimd.dma_start(out=Xq[:, 2 * NQ:], in_=xqi[:, 2 * NQ:])
            nc.gpsimd.dma_start(out=X[:, 2 * NK:], in_=xc[:, 2 * NK:])

            for _ in range(2):
                pL0 = psL.tile([128, 1024], f32, tag="psL")
                nc.vector.memset(pL0[:], 0.0)

            ones = cst.tile([128, 32], bf)
            nc.vector.memset(ones[:], 1.0)
            zrow = cst.tile([128, 128], bf)
            nc.vector.memset(zrow[:], 0.0)

            # Q and K channel-major projections, pipelined per block.
            Q = qkp.tile([128, NBLK * NQ], bf)
            K = qkp.tile([128, NBLK * NK], bf)

            def proj(blk):
                pq = psP.tile([128, 512], f32, tag="psP")
                nc.tensor.matmul(out=pq[:, :NQ], lhsT=w_q[:],
                                 rhs=Xq[:, blk * NQ:(blk + 1) * NQ],
                                 start=True, stop=True)
                if blk % 2:
                    nc.scalar.copy(out=Q[:, blk * NQ:(blk + 1) * NQ], in_=pq[:, :NQ])
                else:
                    nc.vector.tensor_copy(Q[:, blk * NQ:(blk + 1) * NQ], pq[:, :NQ])
                for half in range(2):
                    pk = psP.tile([128, 512], f32, tag="psP")
                    sl = slice(blk * NK + half * NQ, blk * NK + (half + 1) * NQ)
                    nc.tensor.matmul(out=pk[:, :NQ], lhsT=w_k[:], rhs=X[:, sl],
                                     start=True, stop=True)
                    if half:
                        nc.scalar.copy(out=K[:, sl], in_=pk[:, :NQ])
                    else:
                        nc.vector.tensor_copy(K[:, sl], pk[:, :NQ])

            proj(0)

            pending_tail = [None, None]

            def flush_tail(idx):
                if pending_tail[idx] is not None:
                    pending_tail[idx]()
                    pending_tail[idx] = None

            for blk in range(NBLK):
                # --- V^T production: one matmul per column chunk ---
                VT = vtp.tile([128, NC * 128], bf, tag="vt")
                pv = psP.tile([128, 512], f32, tag="psP")
                for c in range(4):
                    pc = 12 * CHUNKS[c][1]
                    nc.tensor.matmul(
                        out=pv[:pc, c * 128:(c + 1) * 128],
                        lhsT=keys_ap(X, blk * NK, c),
                        rhs=w_v[:], start=True, stop=True)
                nc.vector.tensor_copy(VT[:, :512], pv[:, :512])
                pv2 = psP.tile([128, 512], f32, tag="psP")
                nc.tensor.matmul(
                    out=pv2[:96, :128],
                    lhsT=keys_ap(X, blk * NK, 4),
                    rhs=w_v[:], start=True, stop=True)
                nc.scalar.copy(out=VT[:96, 512:640], in_=pv2[:96, :128])

                attnT = attp.tile([128, 4 * NC * SLOT], bf, tag="att")
                pO = psO.tile([128, 512], f32, tag="psO")
                pS = psO.tile([128, 512], f32, tag="psS")
                if blk == 0:
                    # later blocks are zeroed inside the previous tail
                    nc.vector.memset(pO[:, :NQ], 0.0)
                    nc.vector.memset(pS[:, :NQ], 0.0)

                def unit(c, hp, blk=blk, attnT=attnT):
                    """logits + exp + mask for heads {2hp,2hp+1}, chunk c."""
                    kc0, ncols, qlo, qw = CHUNKS[c]
                    pc, nf = 12 * ncols, CR * qw
                    pL = psL.tile([128, 1024], f32, tag="psL")
                    qv = Q[:, blk * NQ:(blk + 1) * NQ].rearrange(
                        "p (r w) -> p r w", r=CR)
                    for hh in range(2):
                        h = 2 * hp + hh
                        nc.tensor.matmul(
                            out=pL[0:pc, 512 * hh: 512 * hh + nf],
                            lhsT=keys_ap(K[32 * h:32 * h + 32], blk * NK, c),
                            rhs=qv[32 * h:32 * h + 32, :, qlo:qlo + qw],
                            start=True, stop=True,
                            tile_position=(32 * h, 0),
                        )
                    src = pL[:pc].rearrange("p (h n) -> p h n", h=2)[:, :, :nf]
                    att2 = attnT[:pc].rearrange("p (h c n) -> p h c n",
                                                h=4, c=NC)[:, 2 * hp:2 * hp + 2,
                                                           c, :nf]
                    nc.scalar.activation(
                        out=att2, in_=src,
                        func=mybir.ActivationFunctionType.Exp,
                        bias=mbias[0:pc, blk * NC + c: blk * NC + c + 1],
                        scale=SCALE,
                    )
                    wm2 = WM[:pc].rearrange("p (h c n) -> p h c n",
                                            h=4, c=NC)[:, 2 * hp:2 * hp + 2,
                                                       c, :nf]
                    nc.vector.tensor_mul(out=att2, in0=att2, in1=wm2)

                def phase2(c, hp, blk=blk, attnT=attnT, pO=pO, pS=pS, VT=VT):
                    kc0, ncols, qlo, qw = CHUNKS[c]
                    pc, nf = 12 * ncols, CR * qw
                    po_v = pO[:, :NQ].rearrange("p (r w) -> p r w", r=CR)
                    ps_v = pS[:, :NQ].rearrange("p (r w) -> p r w", r=CR)
                    for hh in range(2):
                        h = 2 * hp + hh
                        rhs = attnT[0:pc, (h * NC + c) * SLOT:
                                    (h * NC + c) * SLOT + nf]
                        nc.tensor.matmul(
                            out=po_v[32 * h:32 * h + 32, :, qlo:qlo + qw],
                            lhsT=VT[0:pc, c * 128 + 32 * h:
                                    c * 128 + 32 * h + 32],
                            rhs=rhs, start=False,
                            stop=(c == NC - 1 and hp == 1),
                            tile_position=(0, 32 * h),
                        )
                        nc.tensor.matmul(
                            out=ps_v[32 * h:32 * h + 32, :, qlo:qlo + qw],
                            lhsT=ones[0:pc, :],
                            rhs=rhs, start=False,
                            stop=(c == NC - 1 and hp == 1),
                            tile_position=(0, 32 * h),
                        )

                units = [(c, hp) for c in range(NC) for hp in range(2)]
                for i, (c, hp) in enumerate(units):
                    unit(c, hp)
                    if i == 1:
                        flush_tail(0)     # normalize chain of prev block
                    if i == 5:
                        flush_tail(1)     # projection + store of prev block
                    if i == 6 and blk + 1 < NBLK:
                        proj(blk + 1)
                    if i >= 3:
                        phase2(*units[i - 3])
                for j in (7, 8, 9):
                    phase2(*units[j])

                onrm = oev.tile([128, NQ], bf, tag="onrm")

                def tail_a(blk=blk, pO=pO, pS=pS, onrm=onrm):
                    # 1/S = exp(-ln S) on the scalar engine: ~2.5x cheaper
                    # than DVE reciprocal and off the loaded vector engine.
                    lns = oev.tile([128, NQ], f32, tag="lns")
                    nc.scalar.activation(
                        out=lns[:], in_=pS[:, :NQ],
                        func=mybir.ActivationFunctionType.Ln)
                    nc.vector.memset(pS[:, :NQ], 0.0)   # ready for next blk
                    rcp = oev.tile([128, NQ], f32, tag="rcp")
                    nc.scalar.activation(
                        out=rcp[:], in_=lns[:],
                        func=mybir.ActivationFunctionType.Exp, scale=-1.0)
                    nc.vector.tensor_mul(out=onrm[:], in0=pO[:, :NQ],
                                         in1=rcp[:])
                    nc.vector.memset(pO[:, :NQ], 0.0)   # ready for next blk

                def tail_b(blk=blk, onrm=onrm):
                    pF = psP.tile([128, 512], f32, tag="psP")
                    nc.tensor.matmul(out=pF[:, :NQ], lhsT=w_p[:], rhs=onrm[:],
                                     start=True, stop=True)
                    osb = oev.tile([128, NQ], f32, tag="osb")
                    nc.vector.tensor_copy(osb[:], pF[:, :NQ])
                    nc.gpsimd.dma_start(out=out[:, blk * NQ:(blk + 1) * NQ],
                                        in_=osb[:])

                pending_tail[0] = tail_a
                pending_tail[1] = tail_b
            flush_tail(0)
            flush_tail(1)

    _split_multi_waits(nc)
    return nc


def _split_multi_waits(nc):
    """This walrus build rejects >1 sem wait per instruction: move extra
    waits onto dedicated single-wait NoOps inserted just before."""
    import copy
    from concourse import mybir

    tmpl = nc.sync.nop(nofuse=True, hint="wsplit_template").ins
    bb0 = nc.cur_bb.bb
    bb0.instructions = [i for i in bb0.instructions if i.name != tmpl.name]
    tmpl = copy.deepcopy(tmpl)

    ctr = 0
    for f in nc.m.functions:
        for bb in f.blocks:
            insts = list(bb.instructions)
            new, changed = [], False
            for inst in insts:
                si = getattr(inst, "sync_info", None)
                waits = list(si.on_wait) if si is not None and si.on_wait else []
                if len(waits) > 1:
                    for w in waits[:-1]:
                        ctr += 1
                        nop = copy.deepcopy(tmpl)
                        nop.name = f"I-wsplit{ctr}"
                        nop.engine = inst.engine
                        nop.sync_info = mybir.SyncInfo(on_wait=[w], on_update=[])
                        new.append(nop)
                    si.on_wait = [waits[-1]]
                    changed = True
                new.append(inst)
            if changed:
                bb.instructions = new


def _chunk_key_index(c):
    """key indices (r*48+kc) of chunk c, row-major, as used on-device."""
    kc0, ncols, _, _ = CHUNKS[c]
    rr = np.arange(KR)[:, None]
    cc = np.arange(kc0, kc0 + ncols)[None, :]
    return (rr * W2 + cc).reshape(-1)


def _host_prep(x, m):
    import ml_dtypes
    bfd = ml_dtypes.bfloat16
    key_perm = np.concatenate([_chunk_key_index(c) for c in range(NC)])
    xs, xqs, ms = [], [], []
    for k in range(CORES):
        r0 = 12 * k - 6
        xpad = np.zeros((B, C, 24, W), np.float32)
        mpad = np.zeros((B, 1, 24, W), np.int32)
        lo, hi = max(0, r0), min(H, r0 + 24)
        xpad[:, :, lo - r0:hi - r0] = x[:, :, lo:hi]
        mpad[:, :, lo - r0:hi - r0] = m[:, :, lo:hi]
        xcs = xpad.reshape(B, C, KR, 2, W2, 2).transpose(1, 0, 3, 5, 2, 4)
        xcs = xcs.reshape(C, NBLK, NK)
        xq = np.ascontiguousarray(
            xcs[:, :, 144:144 + NQ].reshape(C, NBLK * NQ).astype(bfd))
        xck = np.ascontiguousarray(
            xcs[:, :, key_perm].reshape(C, NBLK * NK).astype(bfd))
        mc = mpad.reshape(B, 1, KR, 2, W2, 2).transpose(1, 0, 3, 5, 2, 4)
        mc = mc.reshape(B, 4, NK)
        mb = np.zeros((128, NBLK * NC), np.float32)
        for b in range(B):
            for cs in range(4):
                for c in range(NC):
                    idx = _chunk_key_index(c)
                    mb[:len(idx), (b * 4 + cs) * NC + c] = np.where(
                        mc[b, cs, idx] > 0, 0.0, MBIAS)
        xs.append(xck)
        xqs.append(xq)
        ms.append(np.ascontiguousarray(mb))
    return xs, xqs, ms


def _host_win():
    """[128, 4*NC*SLOT] bf16: 0/1 win mask, chunk-key partition order,
    4 identical head copies; slot layout (qr, qc-qlo)."""
    import ml_dtypes
    wm = np.zeros((128, 4, NC, SLOT), np.float32)
    for c, (kc0, ncols, qlo, qw) in enumerate(CHUNKS):
        rr = np.arange(KR)[:, None]          # key rows
        cc = np.arange(kc0, kc0 + ncols)[None, :]
        kr = np.repeat(rr, ncols, 1).reshape(-1)[:, None]   # [pc,1]
        kc = np.repeat(cc, KR, 0).reshape(-1)[:, None]
        qr = np.arange(CR)[None, :, None]
        qc = np.arange(qlo, qlo + qw)[None, None, :]
        win = ((kr[:, :, None] - qr >= 0) & (kr[:, :, None] - qr <= 6)
               & (np.abs(kc[:, :, None] - qc) <= 3))
        pc, nf = KR * ncols, CR * qw
        wm[:pc, :, c, :nf] = win.reshape(pc, nf)[:, None, :]
    return np.ascontiguousarray(
        wm.reshape(128, 4 * NC * SLOT).astype(ml_dtypes.bfloat16))


def kernel(x, m, Wq, Wk, Wv, Wp):
    global _prog
    import ml_dtypes
    from concourse.bass_utils import run_bass_kernel_spmd

    bfd = ml_dtypes.bfloat16
    x = np.asarray(x, dtype=np.float32)
    m = np.asarray(m, dtype=np.int32)
    if _prog is None:
        _prog = _build_program()
    nc = _prog

    xs, xqs, ms = _host_prep(x, m)
    wmask = _host_win()
    base = {
        "winm": wmask,
        "wq": np.ascontiguousarray(np.asarray(Wq, np.float32).T.astype(bfd)),
        "wk": np.ascontiguousarray(np.asarray(Wk, np.float32).T.astype(bfd)),
        "wv": np.ascontiguousarray(np.asarray(Wv, np.float32).T.astype(bfd)),
        "wp": np.ascontiguousarray(np.asarray(Wp, np.float32).T.astype(bfd)),
    }
    in_maps = [{**base, "xc": xs[k], "xq": xqs[k], "mb": ms[k]}
               for k in range(CORES)]
    res = run_bass_kernel_spmd(nc, in_maps, list(range(CORES)))

    full = np.zeros((B, C, H, W), np.float32)
    for k in range(CORES):
        oc = res.results[k]["out"].reshape(C, B, 2, 2, CR, W2)
        o = oc.transpose(1, 0, 4, 2, 5, 3).reshape(B, C, 12, 96)
        full[:, :, 12 * k:12 * k + 12, :] = o
    return full


# revision 45
# speedup vs baseline: 1.0293x; 1.0046x over previous
"""Dilated (dil=2) 7x7 window self-attention, 4 heads x 32 dim, on 8 trn2 cores.

Strategy: spatial sharding over image rows (12 rows/core, 6-row halo).
Inside each core, the dilation-2 window decomposes the image into 4
cosets (row/col parity); within a coset the attention is a dense 7x7
window on a 48x48 grid (12 key rows x 48 cols vs 6 query rows x 48
cols per (batch, coset) block).  All tensors are channel-major
[128, pix]; logits are computed transposed [nk, nq] per block so both
attention einsums are matmuls without transposes.  All matmuls run in
bf16 (1 PE cycle/row), accumulation in fp32 PSUM.

Key chunking is by full-height COLUMN slabs: a chunk is all 12 key
rows x ~10 key cols (<=120 partitions).  A chunk's in-window queries
are all 6 query rows x (cols +-3) — so each query is touched by only
~1.5 chunks, vs 4 with row-pair chunking.  That shrinks the streamed
free size of the logits / attn@V / denominator matmuls and the exp /
mask element count by ~2.6x.

  K^T Q  : per (chunk, head) one [32,120]-lhsT matmul (keys via a 2D
           strided AP over the row-major key layout).
  softmax: unnormalized exp (no max-subtraction; logits are tiny) with
           the key-pixel mask bias (-60 per masked key) folded into the
           ACT exp bias, written directly in bf16; out-of-window pairs
           zeroed by one bf16 multiply per (chunk, head-pair) with a
           precomputed 0/1 window tensor; softmax denominators come
           from a ones-weight matmul and are divided out after attn@V.
  attn@V : col-tiled (4 heads) matmuls accumulating into a 2D-strided
           PSUM region (query rows x cols), PSUM zeroed per block by a
           rank-1 zero matmul with start=True.
"""

import numpy as np

HEADS, D, WIN, DIL = 4, 32, 7, 2
B, C, H, W = 2, 128, 96, 96
CORES, RPC = 8, 12
CR, KR, W2 = 6, 12, 48            # coset query rows / key rows (halo) / cols
NQ, NK = CR * W2, KR * W2         # 288, 576
NBLK = B * 4                      # (batch, coset) blocks per core
SCALE = float(1.0 / np.sqrt(D))
MBIAS = -60.0

# full-height column chunks: (key col0, ncols, query col lo, query width)
CHUNKS = [(0, 10, 0, 13), (10, 10, 7, 16), (20, 10, 17, 16),
          (30, 10, 27, 16), (40, 8, 37, 11)]
NC = len(CHUNKS)
SLOT = 96                         # attnT slot per chunk (max 6*16)
CHUNK_PC = [12 * c[1] for c in CHUNKS]            # keys per chunk
CHUNK_OFF = [sum(CHUNK_PC[:i]) for i in range(NC)]  # chunk-major key offset

_prog = None


def _build_program():
    import concourse.bass as bass
    import concourse.tile as tile
    from concourse import mybir

    nc = bass.Bass("TRN2", target_bir_lowering=False, debug=False,
                   num_devices=CORES)
    f32 = mybir.dt.float32
    bf = mybir.dt.bfloat16
    xc = nc.dram_tensor("xc", [128, NBLK * NK], bf, kind="ExternalInput").ap()
    xqi = nc.dram_tensor("xq", [128, NBLK * NQ], bf,
                         kind="ExternalInput").ap()
    mb_i = nc.dram_tensor("mb", [128, NBLK * NC], f32,
                          kind="ExternalInput").ap()
    winm = nc.dram_tensor("winm", [128, 4 * NC * SLOT], bf,
                          kind="ExternalInput").ap()
    wq = nc.dram_tensor("wq", [128, 128], bf, kind="ExternalInput").ap()
    wk = nc.dram_tensor("wk", [128, 128], bf, kind="ExternalInput").ap()
    wv = nc.dram_tensor("wv", [128, 128], bf, kind="ExternalInput").ap()
    wp = nc.dram_tensor("wp", [128, 128], bf, kind="ExternalInput").ap()
    out = nc.dram_tensor("out", [128, NBLK * NQ], f32,
                         kind="ExternalOutput").ap()

    def keys_ap(t, base, c):
        """contiguous chunk-major key slice (host pre-permutes keys)."""
        return t[:, base + CHUNK_OFF[c]: base + CHUNK_OFF[c] + CHUNK_PC[c]]

    with tile.TileContext(nc) as tc:
        with tc.tile_pool(name="cst", bufs=1) as cst, \
             tc.tile_pool(name="big", bufs=1) as big, \
             tc.tile_pool(name="qk", bufs=1) as qkp, \
             tc.tile_pool(name="vt", bufs=2) as vtp, \
             tc.tile_pool(name="att", bufs=2) as attp, \
             tc.tile_pool(name="oev", bufs=3) as oev, \
             tc.tile_pool(name="psL", bufs=2, space="PSUM") as psL, \
             tc.tile_pool(name="psO", bufs=1, space="PSUM") as psO, \
             tc.tile_pool(name="psP", bufs=2, space="PSUM") as psP:

            w_q = cst.tile([128, 128], bf)
            nc.gpsimd.dma_start(out=w_q[:], in_=wq[:])
            w_k = cst.tile([128, 128], bf)
            nc.gpsimd.dma_start(out=w_k[:], in_=wk[:])
            w_v = cst.tile([128, 128], bf)
            nc.gpsimd.dma_start(out=w_v[:], in_=wv[:])
            w_p = cst.tile([128, 128], bf)
            nc.gpsimd.dma_start(out=w_p[:], in_=wp[:])

            # block-0 inputs + masks first so compute starts early, then
            # the rest of X/Xq as bulk transfers.
            X = big.tile([128, NBLK * NK], bf)
            Xq = big.tile([128, NBLK * NQ], bf)
            WM = big.tile([128, 4 * NC * SLOT], bf)   # win mask, (h,c)-major
            mbias = cst.tile([128, NBLK * NC], f32)
            nc.sync.dma_start(out=Xq[:, :NQ], in_=xqi[:, :NQ])
            nc.sync.dma_start(out=X[:, :NK], in_=xc[:, :NK])
            nc.scalar.dma_start(out=mbias[:], in_=mb_i[:])
            nc.scalar.dma_start(out=WM[:], in_=winm[:])
            nc.gpsimd.dma_start(out=Xq[:, NQ:2 * NQ], in_=xqi[:, NQ:2 * NQ])
            nc.gpsimd.dma_start(out=X[:, NK:2 * NK], in_=xc[:, NK:2 * NK])
            nc.gpsimd.dma_start(out=Xq[:, 2 * NQ:], in_=xqi[:, 2 * NQ:])
            nc.gpsimd.dma_start(out=X[:, 2 * NK:], in_=xc[:, 2 * NK:])

            for _ in range(2):
                pL0 = psL.tile([128, 1024], f32, tag="psL")
                nc.vector.memset(pL0[:], 0.0)

            ones = cst.tile([128, 32], bf)
            nc.vector.memset(ones[:], 1.0)

            # Q and K channel-major projections, pipelined per block.
            Q = qkp.tile([128, NBLK * NQ], bf)
            K = qkp.tile([128, NBLK * NK], bf)

            def proj(blk):
                pq = psP.tile([128, 512], f32, tag="psP")
                nc.tensor.matmul(out=pq[:, :NQ], lhsT=w_q[:],
                                 rhs=Xq[:, blk * NQ:(blk + 1) * NQ],
                                 start=True, stop=True)
                if blk % 2:
                    nc.scalar.copy(out=Q[:, blk * NQ:(blk + 1) * NQ], in_=pq[:, :NQ])
                else:
                    nc.vector.tensor_copy(Q[:, blk * NQ:(blk + 1) * NQ], pq[:, :NQ])
                for half in range(2):
                    pk = psP.tile([128, 512], f32, tag="psP")
                    sl = slice(blk * NK + half * NQ, blk * NK + (half + 1) * NQ)
                    nc.tensor.matmul(out=pk[:, :NQ], lhsT=w_k[:], rhs=X[:, sl],
                                     start=True, stop=True)
                    if half:
                        nc.scalar.copy(out=K[:, sl], in_=pk[:, :NQ])
                    else:
                        nc.vector.tensor_copy(K[:, sl], pk[:, :NQ])

            proj(0)

            pending_tail = [None, None]

            def flush_tail(idx):
                if pending_tail[idx] is not None:
                    pending_tail[idx]()
                    pending_tail[idx] = None

            for blk in range(NBLK):
                # --- V^T production: one matmul per column chunk ---
                VT = vtp.tile([128, NC * 128], bf, tag="vt")
                pv = psP.tile([128, 512], f32, tag="psP")
                for c in range(4):
                    pc = 12 * CHUNKS[c][1]
                    nc.tensor.matmul(
                        out=pv[:pc, c * 128:(c + 1) * 128],
                        lhsT=keys_ap(X, blk * NK, c),
                        rhs=w_v[:], start=True, stop=True)
                nc.vector.tensor_copy(VT[:, :512], pv[:, :512])
                pv2 = psP.tile([128, 512], f32, tag="psP")
                nc.tensor.matmul(
                    out=pv2[:96, :128],
                    lhsT=keys_ap(X, blk * NK, 4),
                    rhs=w_v[:], start=True, stop=True)
                nc.scalar.copy(out=VT[:96, 512:640], in_=pv2[:96, :128])

                attnT = attp.tile([128, 4 * NC * SLOT], bf, tag="att")
                pO = psO.tile([128, 512], f32, tag="psO")
                pS = psO.tile([128, 512], f32, tag="psS")
                if blk == 0:
                    # later blocks are zeroed inside the previous tail
                    nc.vector.memset(pO[:, :NQ], 0.0)
                    nc.vector.memset(pS[:, :NQ], 0.0)

                def unit(c, hp, blk=blk, attnT=attnT):
                    """logits + exp + mask for heads {2hp,2hp+1}, chunk c."""
                    kc0, ncols, qlo, qw = CHUNKS[c]
                    pc, nf = 12 * ncols, CR * qw
                    pL = psL.tile([128, 1024], f32, tag="psL")
                    qv = Q[:, blk * NQ:(blk + 1) * NQ].rearrange(
                        "p (r w) -> p r w", r=CR)
                    for hh in range(2):
                        h = 2 * hp + hh
                        nc.tensor.matmul(
                            out=pL[0:pc, 512 * hh: 512 * hh + nf],
                            lhsT=keys_ap(K[32 * h:32 * h + 32], blk * NK, c),
                            rhs=qv[32 * h:32 * h + 32, :, qlo:qlo + qw],
                            start=True, stop=True,
                            tile_position=(32 * h, 0),
                        )
                    src = pL[:pc].rearrange("p (h n) -> p h n", h=2)[:, :, :nf]
                    att2 = attnT[:pc].rearrange("p (h c n) -> p h c n",
                                                h=4, c=NC)[:, 2 * hp:2 * hp + 2,
                                                           c, :nf]
                    nc.scalar.activation(
                        out=att2, in_=src,
                        func=mybir.ActivationFunctionType.Exp,
                        bias=mbias[0:pc, blk * NC + c: blk * NC + c + 1],
                        scale=SCALE,
                    )
                    wm2 = WM[:pc].rearrange("p (h c n) -> p h c n",
                                            h=4, c=NC)[:, 2 * hp:2 * hp + 2,
                                                       c, :nf]
                    nc.vector.tensor_mul(out=att2, in0=att2, in1=wm2)

                def phase2(c, hp, blk=blk, attnT=attnT, pO=pO, pS=pS, VT=VT):
                    kc0, ncols, qlo, qw = CHUNKS[c]
                    pc, nf = 12 * ncols, CR * qw
                    po_v = pO[:, :NQ].rearrange("p (r w) -> p r w", r=CR)
                    ps_v = pS[:, :NQ].rearrange("p (r w) -> p r w", r=CR)
                    for hh in range(2):
                        h = 2 * hp + hh
                        rhs = attnT[0:pc, (h * NC + c) * SLOT:
                                    (h * NC + c) * SLOT + nf]
                        nc.tensor.matmul(
                            out=po_v[32 * h:32 * h + 32, :, qlo:qlo + qw],
                            lhsT=VT[0:pc, c * 128 + 32 * h:
                                    c * 128 + 32 * h + 32],
                            rhs=rhs, start=False,
                            stop=(c == NC - 1 and hp == 1),
                            tile_position=(0, 32 * h),
                        )
                        nc.tensor.matmul(
                            out=ps_v[32 * h:32 * h + 32, :, qlo:qlo + qw],
                            lhsT=ones[0:pc, :],
                            rhs=rhs, start=False,
                            stop=(c == NC - 1 and hp == 1),
                            tile_position=(0, 32 * h),
                        )

                units = [(c, hp) for c in range(NC) for hp in range(2)]
                for i, (c, hp) in enumerate(units):
                    unit(c, hp)
                    if i == 1:
                        flush_tail(0)     # normalize chain of prev block
                    if i == 5:
                        flush_tail(1)     # projection + store of prev block
                    if i == 6 and blk + 1 < NBLK:
                        proj(blk + 1)
                    if i >= 3:
                        phase2(*units[i - 3])
                for j in (7, 8, 9):
                    phase2(*units[j])

                onrm = oev.tile([128, NQ], bf, tag="onrm")

                def tail_a(blk=blk, pO=pO, pS=pS, onrm=onrm):
                    # 1/S = exp(-ln S) on the scalar engine: ~2.5x cheaper
                    # than DVE reciprocal and off the loaded vector engine.
                    lns = oev.tile([128, NQ], f32, tag="lns")
                    nc.scalar.activation(
                        out=lns[:], in_=pS[:, :NQ],
                        func=mybir.ActivationFunctionType.Ln)
                    nc.vector.memset(pS[:, :NQ], 0.0)   # ready for next blk
                    rcp = oev.tile([128, NQ], f32, tag="rcp")
                    nc.scalar.activation(
                        out=rcp[:], in_=lns[:],
                        func=mybir.ActivationFunctionType.Exp, scale=-1.0)
                    nc.vector.tensor_mul(out=onrm[:], in0=pO[:, :NQ],
                                         in1=rcp[:])
                    nc.vector.memset(pO[:, :NQ], 0.0)   # ready for next blk

                def tail_b(blk=blk, onrm=onrm):
                    pF = psP.tile([128, 512], f32, tag="psP")
                    nc.tensor.matmul(out=pF[:, :NQ], lhsT=w_p[:], rhs=onrm[:],
                                     start=True, stop=True)
                    osb = oev.tile([128, NQ], f32, tag="osb")
                    nc.vector.tensor_copy(osb[:], pF[:, :NQ])
                    nc.gpsimd.dma_start(out=out[:, blk * NQ:(blk + 1) * NQ],
                                        in_=osb[:])

                pending_tail[0] = tail_a
                pending_tail[1] = tail_b
            flush_tail(0)
            flush_tail(1)

    _split_multi_waits(nc)
    return nc


def _split_multi_waits(nc):
    """This walrus build rejects >1 sem wait per instruction: move extra
    waits onto dedicated single-wait NoOps inserted just before."""
    import copy
    from concourse import mybir

    tmpl = nc.sync.nop(nofuse=True, hint="wsplit_template").ins
    bb0 = nc.cur_bb.bb
    bb0.instructions = [i for i in bb0.instructions if i.name != tmpl.name]
    tmpl = copy.deepcopy(tmpl)

    ctr = 0
    for f in nc.m.functions:
        for bb in f.blocks:
            insts = list(bb.instructions)
            new, changed = [], False
            for inst in insts:
                si = getattr(inst, "sync_info", None)
                waits = list(si.on_wait) if si is not None and si.on_wait else []
                if len(waits) > 1:
                    for w in waits[:-1]:
                        ctr += 1
                        nop = copy.deepcopy(tmpl)
                        nop.name = f"I-wsplit{ctr}"
                        nop.engine = inst.engine
                        nop.sync_info = mybir.SyncInfo(on_wait=[w], on_update=[])
                        new.append(nop)
                    si.on_wait = [waits[-1]]
                    changed = True
                new.append(inst)
            if changed:
                bb.instructions = new


def _chunk_key_index(c):
    """key indices (r*48+kc) of chunk c, row-major, as used on-device."""
    kc0, ncols, _, _ = CHUNKS[c]
    rr = np.arange(KR)[:, None]
    cc = np.arange(kc0, kc0 + ncols)[None, :]
    return (rr * W2 + cc).reshape(-1)


def _host_prep(x, m):
    import ml_dtypes
    bfd = ml_dtypes.bfloat16
    key_perm = np.concatenate([_chunk_key_index(c) for c in range(NC)])
    xs, xqs, ms = [], [], []
    for k in range(CORES):
        r0 = 12 * k - 6
        xpad = np.zeros((B, C, 24, W), np.float32)
        mpad = np.zeros((B, 1, 24, W), np.int32)
        lo, hi = max(0, r0), min(H, r0 + 24)
        xpad[:, :, lo - r0:hi - r0] = x[:, :, lo:hi]
        mpad[:, :, lo - r0:hi - r0] = m[:, :, lo:hi]
        xcs = xpad.reshape(B, C, KR, 2, W2, 2).transpose(1, 0, 3, 5, 2, 4)
        xcs = xcs.reshape(C, NBLK, NK)
        xq = np.ascontiguousarray(
            xcs[:, :, 144:144 + NQ].reshape(C, NBLK * NQ).astype(bfd))
        xck = np.ascontiguousarray(
            xcs[:, :, key_perm].reshape(C, NBLK * NK).astype(bfd))
        mc = mpad.reshape(B, 1, KR, 2, W2, 2).transpose(1, 0, 3, 5, 2, 4)
        mc = mc.reshape(B, 4, NK)
        mb = np.zeros((128, NBLK * NC), np.float32)
        for b in range(B):
            for cs in range(4):
                for c in range(NC):
                    idx = _chunk_key_index(c)
                    mb[:len(idx), (b * 4 + cs) * NC + c] = np.where(
                        mc[b, cs, idx] > 0, 0.0, MBIAS)
        xs.append(xck)
        xqs.append(xq)
        ms.append(np.ascontiguousarray(mb))
    return xs, xqs, ms


def _host_win():
    """[128, 4*NC*SLOT] bf16: 0/1 win mask, chunk-key partition order,
    4 identical head copies; slot layout (qr, qc-qlo)."""
    import ml_dtypes
    wm = np.zeros((128, 4, NC, SLOT), np.float32)
    for c, (kc0, ncols, qlo, qw) in enumerate(CHUNKS):
        rr = np.arange(KR)[:, None]          # key rows
        cc = np.arange(kc0, kc0 + ncols)[None, :]
        kr = np.repeat(rr, ncols, 1).reshape(-1)[:, None]   # [pc,1]
        kc = np.repeat(cc, KR, 0).reshape(-1)[:, None]
        qr = np.arange(CR)[None, :, None]
        qc = np.arange(qlo, qlo + qw)[None, None, :]
        win = ((kr[:, :, None] - qr >= 0) & (kr[:, :, None] - qr <= 6)
               & (np.abs(kc[:, :, None] - qc) <= 3))
        pc, nf = KR * ncols, CR * qw
        wm[:pc, :, c, :nf] = win.reshape(pc, nf)[:, None, :]
    return np.ascontiguousarray(
        wm.reshape(128, 4 * NC * SLOT).astype(ml_dtypes.bfloat16))


def kernel(x, m, Wq, Wk, Wv, Wp):
    global _prog
    import ml_dtypes
    from concourse.bass_utils import run_bass_kernel_spmd

    bfd = ml_dtypes.bfloat16
    x = np.asarray(x, dtype=np.float32)
    m = np.asarray(m, dtype=np.int32)
    if _prog is None:
        _prog = _build_program()
    nc = _prog

    xs, xqs, ms = _host_prep(x, m)
    wmask = _host_win()
    base = {
        "winm": wmask,
        "wq": np.ascontiguousarray(np.asarray(Wq, np.float32).T.astype(bfd)),
        "wk": np.ascontiguousarray(np.asarray(Wk, np.float32).T.astype(bfd)),
        "wv": np.ascontiguousarray(np.asarray(Wv, np.float32).T.astype(bfd)),
        "wp": np.ascontiguousarray(np.asarray(Wp, np.float32).T.astype(bfd)),
    }
    in_maps = [{**base, "xc": xs[k], "xq": xqs[k], "mb": ms[k]}
               for k in range(CORES)]
    res = run_bass_kernel_spmd(nc, in_maps, list(range(CORES)))

    full = np.zeros((B, C, H, W), np.float32)
    for k in range(CORES):
        oc = res.results[k]["out"].reshape(C, B, 2, 2, CR, W2)
        o = oc.transpose(1, 0, 4, 2, 5, 3).reshape(B, C, 12, 96)
        full[:, :, 12 * k:12 * k + 12, :] = o
    return full
